# revision 12
# baseline (speedup 1.0000x reference)
"""Multi-hot embedding bag kernel for Trainium2 (8 NeuronCores).

Computes, for 5 feature groups g with multi-hot int32 matrices A_g [B, V_g]
and weights W_g [V_g, 64]:
    out = concat_g(norm_g(A_g @ W_g))  with the original module's quirks:
    - "decades" is normalized by its own row-sum AND by the movie row-sum
    - "movies" is never normalized
    - remaining groups are normalized by their own row-sum (rows with sum 0
      are left unnormalized)

v8 strategy: 2D sharding, 4 batch groups x 2 vocab halves. Core c handles
batch rows [(c//2)*512, (c//2+1)*512) and vocab half c%2 of every group.
  - The multi-hot values are exactly {0, 1}, so the host pre-packs each A_g
    slice TRANSPOSED into fp8e4 (0.0 / 1.0 are exact in e4m3) with a
    partition-major chunk layout [128, C, 512]: partition p / chunk c /
    batch col b. No on-device transposes; idx HBM traffic is 4x below
    int32.
  - Weights are host-packed as [W_g | 1] chunks [128, C, 65]; the ones
    column makes the matmul emit row-sums for free. Vocab-halving cuts the
    per-core weight read to 7.8 MB. Movie weights set the output scale
    (never normalized) and stay fp16; all other groups are normalized by
    their row-sums, so their weights ride as fp8e4 scaled by 32 (the 1/32
    folds into the normalization) with negligible output error.
  - Chunks stream in a period-3 [fp8, fp8, mov] interleave so every DMA
    slab carries a near-uniform byte load. Per chunk: ONE matmul
    (lhsT = w chunk [128, 65] stationary, rhs = idx chunk [128, 512] fp8
    moving) accumulating a [65, 512] group result in its own PSUM bank.
  - Group end: the [65, 512] partial bounces SBUF -> internal DRAM, a
    2-rank AllReduce (adjacent-core pairs) sums the two vocab halves, and
    the result is read back, PE-transposed, and normalized. Per-group
    collectives overlap the remaining matmuls; both cores of a pair
    finalize identically (SPMD) and the host keeps the even cores' output.
"""

import math

import numpy as np

import concourse.bass as bass
import concourse.tile as tile
from concourse import bacc, mybir
from concourse.bass_utils import run_bass_kernel_spmd
from concourse.masks import make_identity

B = 2048
LF = 64
FE = LF + 1  # weights + ones column
N_CORES = 8
BG = 4  # batch groups
BR = B // BG  # 512 batch rows per core
NT = BR // 128  # 4 batch tiles of 128
P = 128
W8_SCALE = 32.0  # fp8 weight groups are stored as 32*W to dodge denormals

# (key, idx input name, weight input name, vocab size, output column offset,
#  fp8 weights?)  The fp8-weight groups form one chunk stream, movies the
# other; they are interleaved 2:1. Decades' finalize is deferred until
# movies' row-sum reciprocal exists.
GROUPS = [
    ("dec", "decade_idxs", "W_dec", 12, 0, True),
    ("cat", "category_idxs", "W_cat", 32, 128, True),
    ("per", "person_idxs", "W_per", 100000, 192, True),
    ("com", "company_idxs", "W_com", 20000, 256, True),
    ("mov", "movie_idxs", "W_mov", 60000, 64, False),
]
OUT_COLS = 5 * LF
# chunks per vocab HALF (both halves padded to the same count)
NCHH = [math.ceil(math.ceil(v / P) / 2) for _, _, _, v, _, _ in GROUPS]
CTOT = sum(NCHH)
C8 = sum(n for n, g in zip(NCHH, GROUPS) if g[5])  # fp8-weight chunks
C16 = CTOT - C8  # fp16-weight (movie) chunks

_FP8 = mybir.dt.float8e4
_FP16 = mybir.dt.float16
_FP32 = mybir.dt.float32

MAX_SLAB = 63  # chunks per slab (4 MB idx); all slab sizes multiples of 3
REPLICA_GROUPS = [[0, 1], [2, 3], [4, 5], [6, 7]]


def _chunk_order():
    """Global chunk stream: period-3 [fp8, fp8, mov] interleave, then
    leftovers. Entries: (is8, stream_idx)."""
    order = []
    i8 = i16 = 0
    while i8 < C8 or i16 < C16:
        for _ in range(2):
            if i8 < C8:
                order.append((True, i8))
                i8 += 1
        if i16 < C16:
            order.append((False, i16))
            i16 += 1
    return order


def _slab_plan():
    plan = []
    left = CTOT
    for want in (12, 24, 24):
        take = min(want, left)
        if take:
            plan.append(take)
        left -= take
    while left > 0:
        take = min(MAX_SLAB, left)
        plan.append(take)
        left -= take
    return plan


def _build() -> bass.Bass:
    nc = bacc.Bacc(None, target_bir_lowering=False, num_devices=N_CORES)

    a_dram = nc.dram_tensor("a_all", [P, CTOT * BR], _FP8, kind="ExternalInput")
    w8_dram = nc.dram_tensor("w8", [P, C8 * FE], _FP8, kind="ExternalInput")
    w16_dram = nc.dram_tensor("w16", [P, C16 * FE], _FP16, kind="ExternalInput")
    out = nc.dram_tensor("out", [BR, OUT_COLS], _FP32, kind="ExternalOutput")

    # per-stream chunk -> (group index, is_start, is_stop)
    meta8, meta16 = [], []
    for gi, (n, g) in enumerate(zip(NCHH, GROUPS)):
        m = meta8 if g[5] else meta16
        for j in range(n):
            m.append((gi, j == 0, j == n - 1))
    order = _chunk_order()

    with tile.TileContext(nc) as tc:
        with (
            tc.tile_pool(name="singles", bufs=1) as singles,
            tc.tile_pool(name="apool", bufs=3) as apool,
            tc.tile_pool(name="w8pool", bufs=3) as w8pool,
            tc.tile_pool(name="w16pool", bufs=3) as w16pool,
            tc.tile_pool(name="npool", bufs=4) as npool,
            tc.tile_pool(name="pinp", bufs=2, space="DRAM") as pinp,
            tc.tile_pool(name="poutp", bufs=2, space="DRAM") as poutp,
            tc.tile_pool(name="decp", bufs=1, space="PSUM") as decp,
            tc.tile_pool(name="catp", bufs=1, space="PSUM") as catp,
            tc.tile_pool(name="perp", bufs=1, space="PSUM") as perp,
            tc.tile_pool(name="comp", bufs=1, space="PSUM") as comp,
            tc.tile_pool(name="movp", bufs=1, space="PSUM") as movp,
            tc.tile_pool(name="backp", bufs=1, space="PSUM") as backp,
        ):
            pools = {"dec": decp, "cat": catp, "per": perp,
                     "com": comp, "mov": movp}
            ident32 = singles.tile([P, P], _FP32)
            make_identity(nc, ident32)

            out_sb = singles.tile([P, NT, OUT_COLS], _FP32, name="out_sb")
            rmov = [singles.tile([P, 1], _FP32, name=f"rmov{i}")
                    for i in range(NT)]

            def finalize(gi, accT):
                """Bounce the [65, BR] partial through DRAM, AllReduce the
                two vocab halves, then transpose back and normalize."""
                key, _, _, _, col, is8 = GROUPS[gi]
                accT_sb = npool.tile([FE, BR], _FP32, tag="accsb")
                nc.vector.tensor_copy(accT_sb, accT)
                pin = pinp.tile([FE, BR], _FP32, tag="pin")
                pout = poutp.tile([FE, BR], _FP32, tag="pout")
                nc.sync.dma_start(pin, accT_sb)
                nc.gpsimd.collective_compute(
                    "AllReduce", mybir.AluOpType.add,
                    replica_groups=REPLICA_GROUPS,
                    ins=[pin.opt()], outs=[pout.opt()],
                )
                accR_sb = npool.tile([FE, BR], _FP32, tag="accr")
                nc.scalar.dma_start(accR_sb, pout)
                for bt in range(NT):
                    out2 = backp.tile([P, FE], _FP32, tag="out2")
                    nc.tensor.matmul(
                        out2,
                        lhsT=accR_sb[:, bass.ts(bt, P)],
                        rhs=ident32[:FE, :FE],
                        start=True, stop=True,
                    )
                    s = npool.tile([P, 1], _FP32, tag="s")
                    nc.vector.tensor_scalar_max(s, out2[:, LF:FE], 1.0)
                    if is8:
                        # emb rows carry 32*W; divide by 32*max(sum, 1)
                        nc.vector.tensor_scalar_mul(s, s, W8_SCALE)
                    nc.vector.reciprocal(s, s)
                    if key == "mov":
                        # movies stay unnormalized; stash 1/max(sum,1) for
                        # the decades double-normalization
                        nc.vector.tensor_copy(rmov[bt], s)
                        nc.scalar.copy(out_sb[:, bt, col:col + LF],
                                       out2[:, :LF])
                    else:
                        if key == "dec":
                            nc.vector.tensor_mul(s, s, rmov[bt])
                        nc.vector.tensor_scalar_mul(
                            out_sb[:, bt, col:col + LF], out2[:, :LF], s)

            accs = {}  # group index -> live PSUM tile
            c0 = 0
            for ch in _slab_plan():
                slab = order[c0:c0 + ch]
                n8 = sum(1 for is8, _ in slab if is8)
                n16 = ch - n8
                s8 = next((si for is8, si in slab if is8), 0)
                s16 = next((si for is8, si in slab if not is8), 0)

                a_sb = apool.tile([P, MAX_SLAB, BR], _FP8, tag="a")
                nc.sync.dma_start(
                    a_sb[:, :ch, :],
                    a_dram[:, c0 * BR:(c0 + ch) * BR].rearrange(
                        "p (c b) -> p c b", b=BR),
                )
                w8_sb = w16_sb = None
                if n8:
                    w8_sb = w8pool.tile([P, MAX_SLAB, FE], _FP8, tag="w8")
                    nc.scalar.dma_start(
                        w8_sb[:, :n8, :],
                        w8_dram[:, s8 * FE:(s8 + n8) * FE].rearrange(
                            "p (c f) -> p c f", f=FE),
                    )
                if n16:
                    w16_sb = w16pool.tile([P, MAX_SLAB, FE], _FP16, tag="w16")
                    nc.scalar.dma_start(
                        w16_sb[:, :n16, :],
                        w16_dram[:, s16 * FE:(s16 + n16) * FE].rearrange(
                            "p (c f) -> p c f", f=FE),
                    )

                for j, (is8, si) in enumerate(slab):
                    gi, is_start, is_stop = (meta8 if is8 else meta16)[si]
                    key = GROUPS[gi][0]
                    if is_start:
                        accs[gi] = pools[key].tile([FE, BR], _FP32, tag="acc",
                                                   name=f"acc_{key}")
                    w_sb = w8_sb if is8 else w16_sb
                    wj = si - (s8 if is8 else s16)
                    nc.tensor.matmul(
                        accs[gi],
                        lhsT=w_sb[:, wj, :],
                        rhs=a_sb[:, j, :],
                        start=is_start,
                        stop=is_stop,
                    )
                    if is_stop and key != "dec":
                        finalize(gi, accs[gi])
                c0 += ch

            # movies finalized above (sets rmov); decades deferred to here
            finalize(0, accs[0])

            nc.sync.dma_start(
                out.rearrange("(t p) f -> p t f", t=NT), out_sb)

    nc.finalize()
    return nc


_NC_CACHE: bass.Bass | None = None


def _get_nc() -> bass.Bass:
    global _NC_CACHE
    if _NC_CACHE is None:
        _NC_CACHE = _build()
    return _NC_CACHE


def _pack_weights_half(w: np.ndarray, vh: int, fp8: bool) -> np.ndarray:
    """Vocab half vh of [V, 64] fp32 -> [128, C*65] (fp16, or fp8 scaled by
    32) with ones column and zero padding, chunk-major: chunk c /
    partition p / feature f = row vh*C*128 + c*128 + p of [W | 1]."""
    import ml_dtypes

    v = w.shape[0]
    c = math.ceil(math.ceil(v / P) / 2)  # chunks per half
    scale = W8_SCALE if fp8 else 1.0
    we = np.concatenate([w.astype(np.float32) * scale,
                        np.ones((v, 1), np.float32)], axis=1)
    if 2 * c * P > v:
        we = np.concatenate(
            [we, np.zeros((2 * c * P - v, FE), np.float32)], axis=0)
    we = we[vh * c * P:(vh + 1) * c * P]
    we = we.astype(ml_dtypes.float8_e4m3 if fp8 else np.float16)
    return np.ascontiguousarray(
        we.reshape(c, P, FE).transpose(1, 0, 2).reshape(P, c * FE))


def _pack_idx_group(x: np.ndarray) -> list[list[np.ndarray]]:
    """[B, V] int32 {0,1} -> per (bg, vh): [128, C, 512] uint8 fp8e4 bit
    patterns, element (p, c, b) = 0x38 * x[bg*512 + b, (vh*C + c)*128 + p]."""
    v = x.shape[1]
    c = math.ceil(math.ceil(v / P) / 2)
    xb = (x != 0).astype(np.uint8) * np.uint8(0x38)
    if 2 * c * P > v:
        xb = np.concatenate(
            [xb, np.zeros((B, 2 * c * P - v), np.uint8)], axis=1)
    # [B, 2C*128] -> [4 bg, 512 b, 2C, 128 p] -> [4, 128, 2C, 512]
    t = np.ascontiguousarray(
        xb.reshape(BG, BR, 2 * c, P).transpose(0, 3, 2, 1))
    return [[t[bg, :, vh * c:(vh + 1) * c, :] for vh in range(2)]
            for bg in range(BG)]


def kernel(**inputs: np.ndarray) -> np.ndarray:
    import os

    import ml_dtypes

    nc = _get_nc()

    w8 = {vh: np.concatenate(
        [_pack_weights_half(np.asarray(inputs[wn]), vh, True)
         for _, _, wn, _, _, f8 in GROUPS if f8], axis=1) for vh in range(2)}
    w16 = {vh: np.concatenate(
        [_pack_weights_half(np.asarray(inputs[wn]), vh, False)
         for _, _, wn, _, _, f8 in GROUPS if not f8], axis=1)
        for vh in range(2)}
    a_parts = [_pack_idx_group(np.asarray(inputs[an]))
               for _, an, _, _, _, _ in GROUPS]

    # global chunk permutation: stream chunks in the interleaved order
    order = _chunk_order()
    perm = np.empty(CTOT, np.int64)
    for pos, (is8, si) in enumerate(order):
        perm[pos] = si if is8 else C8 + si

    in_maps = []
    for core in range(N_CORES):
        bg, vh = core // 2, core % 2
        a_core = np.concatenate([p[bg][vh] for p in a_parts], axis=1)
        a_core = np.ascontiguousarray(a_core[:, perm, :])
        in_maps.append({
            "a_all": a_core.reshape(P, CTOT * BR).view(ml_dtypes.float8_e4m3),
            "w8": w8[vh],
            "w16": w16[vh],
        })

    trace = bool(int(os.environ.get("EMB_TRACE", "0")))
    res = run_bass_kernel_spmd(nc, in_maps, core_ids=list(range(N_CORES)),
                               trace=trace)
    if trace and res.exec_time_ns is not None:
        print(f"HW exec time: {res.exec_time_ns} ns")
        if res.instructions_and_trace is not None:
            print(f"trace: {res.instructions_and_trace[1]}")

    # pairs (2bg, 2bg+1) hold identical AllReduced outputs; keep the even one
    return np.concatenate([res.results[2 * bg]["out"] for bg in range(BG)],
                          axis=0)


# revision 16
# speedup vs baseline: 1.2905x; 1.2905x over previous
"""Multi-hot embedding bag kernel for Trainium2 (8 NeuronCores, batch-sharded).

Computes, for 5 feature groups g with multi-hot int32 matrices A_g [B, V_g]
and weights W_g [V_g, 64]:
    out = concat_g(norm_g(A_g @ W_g))  with the original module's quirks:
    - "decades" is normalized by its own row-sum AND by the movie row-sum
    - "movies" is never normalized
    - remaining groups are normalized by their own row-sum (rows with sum 0
      are left unnormalized)

v4 strategy (per core, 256 batch rows):
  - The multi-hot values are exactly {0, 1}, so the host pre-packs each A_g
    TRANSPOSED into fp8e4 (0.0 / 1.0 are exact in e4m3) with a
    partition-major chunk layout [128, C, 256]: partition p / chunk c /
    batch col b holds A_g[b, c*128 + p]. No on-device transposes, and idx
    HBM traffic shrinks 4x vs int32.
  - Weights are host-packed as [W_g | 1] chunks [128, C, 65]; the ones
    column makes the matmul emit row-sums for free. The movie weights set
    the output scale (movies are never normalized) and stay fp16; every
    other group's output is divided by its row-sum, so those weights are
    stored fp8e4 scaled by 32 (the 1/32 folds into the normalization) with
    negligible contribution to output error.
  - Chunks are streamed in a period-3 [fp8, fp8, mov] interleave so every
    DMA slab carries a near-uniform byte load; otherwise the movie block
    (2-byte weights) starves the PE. Each group accumulates into its own
    PSUM tile (start/stop per group), so interleaved accumulation is safe.
  - Per chunk: ONE matmul (lhsT = w chunk [128, 65] stationary, rhs = idx
    chunk [128, 256] fp8 moving) accumulating a transposed [65, 256] group
    result in PSUM. Leading slabs are small so the PE starts within ~5 us.
  - Group end: copy PSUM accumulator to SBUF, transpose back on the PE
    (fp32 identity), then normalize with per-row reciprocals.
"""

import math

import numpy as np

import concourse.bass as bass
import concourse.tile as tile
from concourse import bacc, mybir
from concourse.bass_utils import run_bass_kernel_spmd
from concourse.masks import make_identity

B = 2048
LF = 64
FE = LF + 1  # weights + ones column
N_CORES = 8
BPC = B // N_CORES  # 256 batch rows per core
P = 128
W8_SCALE = 32.0  # fp8 weight groups are stored as 32*W to dodge denormals

# (key, idx input name, weight input name, vocab size, output column offset,
#  fp8 weights?)  The fp8-weight groups form one chunk stream, movies the
# other; they are interleaved 2:1. Decades' finalize is deferred until
# movies' row-sum reciprocal exists.
GROUPS = [
    ("dec", "decade_idxs", "W_dec", 12, 0, True),
    ("cat", "category_idxs", "W_cat", 32, 128, True),
    ("per", "person_idxs", "W_per", 100000, 192, True),
    ("com", "company_idxs", "W_com", 20000, 256, True),
    ("mov", "movie_idxs", "W_mov", 60000, 64, False),
]
OUT_COLS = 5 * LF
NCH = [math.ceil(v / P) for _, _, _, v, _, _ in GROUPS]
CTOT = sum(NCH)
C8 = sum(n for n, g in zip(NCH, GROUPS) if g[5])  # fp8-weight chunks
C16 = CTOT - C8  # fp16-weight (movie) chunks

_FP8 = mybir.dt.float8e4
_FP16 = mybir.dt.float16
_FP32 = mybir.dt.float32

MAX_SLAB = 126  # chunks per slab; all slab sizes are multiples of 3


def _chunk_order():
    """Global chunk stream: period-3 [fp8, fp8, mov] interleave, then
    leftovers. Entries: (is8, stream_idx) where stream_idx indexes the
    fp8 or fp16 chunk stream (each stream keeps its groups in GROUPS
    order)."""
    order = []
    i8 = i16 = 0
    while i8 < C8 or i16 < C16:
        for _ in range(2):
            if i8 < C8:
                order.append((True, i8))
                i8 += 1
        if i16 < C16:
            order.append((False, i16))
            i16 += 1
    return order


def _slab_plan():
    plan = []
    left = CTOT
    for want in (24, 24, 48):
        take = min(want, left)
        if take:
            plan.append(take)
        left -= take
    while left > 0:
        take = min(MAX_SLAB, left)
        plan.append(take)
        left -= take
    return plan


def _build() -> bass.Bass:
    nc = bacc.Bacc(None, target_bir_lowering=False)

    a_dram = nc.dram_tensor("a_all", [P, CTOT * BPC], _FP8, kind="ExternalInput")
    w8_dram = nc.dram_tensor("w8", [P, C8 * FE], _FP8, kind="ExternalInput")
    w16_dram = nc.dram_tensor("w16", [P, C16 * FE], _FP16, kind="ExternalInput")
    out = nc.dram_tensor("out", [BPC, OUT_COLS], _FP32, kind="ExternalOutput")

    # per-stream chunk -> (group index, is_start, is_stop)
    meta8, meta16 = [], []
    for gi, (n, g) in enumerate(zip(NCH, GROUPS)):
        m = meta8 if g[5] else meta16
        for j in range(n):
            m.append((gi, j == 0, j == n - 1))
    order = _chunk_order()

    with tile.TileContext(nc) as tc:
        with (
            tc.tile_pool(name="singles", bufs=1) as singles,
            tc.tile_pool(name="apool", bufs=3) as apool,
            tc.tile_pool(name="w8pool", bufs=3) as w8pool,
            tc.tile_pool(name="w16pool", bufs=3) as w16pool,
            tc.tile_pool(name="npool", bufs=4) as npool,
            tc.tile_pool(name="accp", bufs=3, space="PSUM") as accp,
            tc.tile_pool(name="decp", bufs=1, space="PSUM") as decp,
            tc.tile_pool(name="movp", bufs=1, space="PSUM") as movp,
            tc.tile_pool(name="backp", bufs=1, space="PSUM") as backp,
        ):
            ident32 = singles.tile([P, P], _FP32)
            make_identity(nc, ident32)

            out_sb = [singles.tile([P, OUT_COLS], _FP32, name=f"out_sb{i}")
                      for i in range(2)]
            rmov = [singles.tile([P, 1], _FP32, name=f"rmov{i}")
                    for i in range(2)]

            def finalize(gi, accT):
                key, _, _, _, col, is8 = GROUPS[gi]
                accT_sb = npool.tile([FE, 2 * P], _FP32, tag="accsb")
                nc.vector.tensor_copy(accT_sb, accT)
                for bt in range(2):
                    out2 = backp.tile([P, FE], _FP32, tag="out2")
                    nc.tensor.matmul(
                        out2,
                        lhsT=accT_sb[:, bass.ts(bt, P)],
                        rhs=ident32[:FE, :FE],
                        start=True, stop=True,
                    )
                    s = npool.tile([P, 1], _FP32, tag="s")
                    nc.vector.tensor_scalar_max(s, out2[:, LF:FE], 1.0)
                    if is8:
                        # emb rows carry 32*W; divide by 32*max(sum, 1)
                        nc.vector.tensor_scalar_mul(s, s, W8_SCALE)
                    nc.vector.reciprocal(s, s)
                    if key == "mov":
                        # movies stay unnormalized; stash 1/max(sum,1) for
                        # the decades double-normalization
                        nc.vector.tensor_copy(rmov[bt], s)
                        nc.scalar.copy(out_sb[bt][:, col:col + LF],
                                       out2[:, :LF])
                    else:
                        if key == "dec":
                            nc.vector.tensor_mul(s, s, rmov[bt])
                        nc.vector.tensor_scalar_mul(
                            out_sb[bt][:, col:col + LF], out2[:, :LF], s)

            accs = {}  # group index -> live PSUM tile
            c0 = 0
            for ch in _slab_plan():
                slab = order[c0:c0 + ch]
                n8 = sum(1 for is8, _ in slab if is8)
                n16 = ch - n8
                s8 = next((si for is8, si in slab if is8), 0)
                s16 = next((si for is8, si in slab if not is8), 0)

                a_sb = apool.tile([P, MAX_SLAB, BPC], _FP8, tag="a")
                nc.sync.dma_start(
                    a_sb[:, :ch, :],
                    a_dram[:, c0 * BPC:(c0 + ch) * BPC].rearrange(
                        "p (c b) -> p c b", b=BPC),
                )
                w8_sb = w16_sb = None
                if n8:
                    w8_sb = w8pool.tile([P, MAX_SLAB, FE], _FP8, tag="w8")
                    nc.scalar.dma_start(
                        w8_sb[:, :n8, :],
                        w8_dram[:, s8 * FE:(s8 + n8) * FE].rearrange(
                            "p (c f) -> p c f", f=FE),
                    )
                if n16:
                    w16_sb = w16pool.tile([P, MAX_SLAB, FE], _FP16, tag="w16")
                    nc.scalar.dma_start(
                        w16_sb[:, :n16, :],
                        w16_dram[:, s16 * FE:(s16 + n16) * FE].rearrange(
                            "p (c f) -> p c f", f=FE),
                    )

                for j, (is8, si) in enumerate(slab):
                    gi, is_start, is_stop = (meta8 if is8 else meta16)[si]
                    key = GROUPS[gi][0]
                    if is_start:
                        pool = {"dec": decp, "mov": movp}.get(key, accp)
                        accs[gi] = pool.tile([FE, 2 * P], _FP32, tag="acc",
                                             name=f"acc_{key}")
                    w_sb = w8_sb if is8 else w16_sb
                    wj = si - (s8 if is8 else s16)
                    nc.tensor.matmul(
                        accs[gi],
                        lhsT=w_sb[:, wj, :],
                        rhs=a_sb[:, j, :],
                        start=is_start,
                        stop=is_stop,
                    )
                    if is_stop and key != "dec":
                        finalize(gi, accs[gi])
                c0 += ch

            # movies finalized above (sets rmov); decades deferred to here
            finalize(0, accs[0])

            for bt in range(2):
                nc.sync.dma_start(out[bt * P:(bt + 1) * P, :], out_sb[bt])

    nc.finalize()
    return nc


_NC_CACHE: bass.Bass | None = None


def _get_nc() -> bass.Bass:
    global _NC_CACHE
    if _NC_CACHE is None:
        _NC_CACHE = _build()
    return _NC_CACHE


def _pack_weights(w: np.ndarray, fp8: bool) -> np.ndarray:
    """[V, 64] fp32 -> [128, C*65] (fp16, or fp8 scaled by 32) with ones
    column and zero row padding, laid out so chunk c / partition p /
    feature f = row c*128+p of [W | 1]."""
    import ml_dtypes

    v = w.shape[0]
    c = math.ceil(v / P)
    scale = W8_SCALE if fp8 else 1.0
    we = np.concatenate([w.astype(np.float32) * scale,
                        np.ones((v, 1), np.float32)], axis=1)
    if c * P > v:
        we = np.concatenate([we, np.zeros((c * P - v, FE), np.float32)], axis=0)
    we = we.astype(ml_dtypes.float8_e4m3 if fp8 else np.float16)
    return np.ascontiguousarray(
        we.reshape(c, P, FE).transpose(1, 0, 2).reshape(P, c * FE))


def _pack_idx_group(x: np.ndarray) -> np.ndarray:
    """[B, V] int32 {0,1} -> [8, 128, C, 256] uint8 fp8e4 bit patterns,
    element (core, p, c, b) = 0x38 * x[core*256 + b, c*128 + p]."""
    v = x.shape[1]
    c = math.ceil(v / P)
    xb = (x != 0).astype(np.uint8) * np.uint8(0x38)
    if c * P > v:
        xb = np.concatenate(
            [xb, np.zeros((B, c * P - v), np.uint8)], axis=1)
    # [B, C*128] -> [8 cores, 256 b, C, 128 p] -> [8, 128, C, 256]
    return np.ascontiguousarray(
        xb.reshape(N_CORES, BPC, c, P).transpose(0, 3, 2, 1))


def kernel(**inputs: np.ndarray) -> np.ndarray:
    import os

    import ml_dtypes

    nc = _get_nc()

    w8 = np.concatenate(
        [_pack_weights(np.asarray(inputs[wn]), True)
         for _, _, wn, _, _, f8 in GROUPS if f8], axis=1)
    w16 = np.concatenate(
        [_pack_weights(np.asarray(inputs[wn]), False)
         for _, _, wn, _, _, f8 in GROUPS if not f8], axis=1)
    a_parts = [_pack_idx_group(np.asarray(inputs[an]))
               for _, an, _, _, _, _ in GROUPS]

    # global chunk permutation: stream chunks in the interleaved order
    order = _chunk_order()
    perm = np.empty(CTOT, np.int64)
    for pos, (is8, si) in enumerate(order):
        perm[pos] = si if is8 else C8 + si

    in_maps = []
    for core in range(N_CORES):
        a_core = np.concatenate([p[core] for p in a_parts], axis=1)
        a_core = np.ascontiguousarray(a_core[:, perm, :])
        in_maps.append({
            "a_all": a_core.reshape(P, CTOT * BPC).view(ml_dtypes.float8_e4m3),
            "w8": w8,
            "w16": w16,
        })

    trace = bool(int(os.environ.get("EMB_TRACE", "0")))
    res = run_bass_kernel_spmd(nc, in_maps, core_ids=list(range(N_CORES)),
                               trace=trace)
    if trace and res.exec_time_ns is not None:
        print(f"HW exec time: {res.exec_time_ns} ns")
        if res.instructions_and_trace is not None:
            print(f"trace: {res.instructions_and_trace[1]}")

    return np.concatenate([r["out"] for r in res.results], axis=0)
